# revision 2
# baseline (speedup 1.0000x reference)
"""Cross-attention Trainium2 kernel (data-parallel over batch, 8 cores).

Reference computation (per batch b):
    q = x @ Wq, k = ctx @ Wk, v = ctx @ Wv         (16 heads x 64)
    out = softmax(q k^T / sqrt(64)) v
    y = out @ Wo + bo

Device dataflow is fully "transposed": the host feeds x^T / ctx^T per core,
the Q/K projections emit q^T / k^T directly, attention scores are computed
as S^T = k_h^T{stationary} x q_h^T{moving} so softmax-exp runs along the
free axis with no on-device transposes, the second attention matmul uses
stationary [v_h | ones] to produce out^T plus the softmax denominator row,
and the output projection emits y^T (host transposes back).  Softmax skips
the max-subtraction (|S| <= ~8 for this problem => exp is safe in fp32).

Matmul operands are bf16 (fp32 PSUM accumulation); softmax denominators are
carried in fp32 via ln/exp reciprocal on the scalar engine.
"""
import os
import sys

sys.path.insert(0, "/opt/trn_rl_repo")

import numpy as np

HEADS = 16
DH = 64
B = 16
NQ = 4096
DQ = 1024
NC = 77
DC = 768
INNER = 1024
SCALE = DH ** -0.5

NCORES = 8
BLOCAL = B // NCORES          # 2 batches per core
BLK = 512                     # q rows per block
NBLK = NQ // BLK
KT_Q = DQ // 128              # 8 contraction tiles for x @ Wq
KT_C = DC // 128              # 6 contraction tiles for ctx @ Wk/Wv
VAUG = DH + 1                 # v columns per head + ones column

_cache = {}


def _build():
    import concourse.bacc as bacc
    import concourse.mybir as mybir
    import concourse.tile as tile

    f32 = mybir.dt.float32
    bf = mybir.dt.bfloat16
    AF = mybir.ActivationFunctionType

    nc = bacc.Bacc(None, target_bir_lowering=False)

    xT = nc.dram_tensor("xT", [BLOCAL, DQ, NQ], bf, kind="ExternalInput")
    ctxT = nc.dram_tensor("ctxT", [BLOCAL, DC, NC], bf, kind="ExternalInput")
    wq = nc.dram_tensor("wq", [DQ, INNER], bf, kind="ExternalInput")
    wk = nc.dram_tensor("wk", [DC, INNER], bf, kind="ExternalInput")
    wv = nc.dram_tensor("wv", [DC, INNER], bf, kind="ExternalInput")
    wo = nc.dram_tensor("wo", [INNER, DQ], bf, kind="ExternalInput")
    bo2 = nc.dram_tensor("bo2", [128, DQ // 128], f32, kind="ExternalInput")
    outT = nc.dram_tensor("outT", [BLOCAL, DQ, NQ], f32, kind="ExternalOutput")

    with tile.TileContext(nc) as tc:
        with (
            tc.tile_pool(name="wp", bufs=1) as wp,
            tc.tile_pool(name="kvp", bufs=2) as kvp,
            tc.tile_pool(name="xtp", bufs=2) as xtp,
            tc.tile_pool(name="qtp", bufs=10) as qtp,
            tc.tile_pool(name="ptp", bufs=6) as ptp,
            tc.tile_pool(name="otzp", bufs=17) as otzp,
            tc.tile_pool(name="izrp", bufs=6) as izrp,
            tc.tile_pool(name="zbp", bufs=2) as zbp,
            tc.tile_pool(name="otp", bufs=10) as otp,
            tc.tile_pool(name="fop", bufs=2) as fop,
            tc.tile_pool(name="qpp", bufs=2, space="PSUM") as qpp,
            tc.tile_pool(name="spp", bufs=2, space="PSUM") as spp,
            tc.tile_pool(name="opp", bufs=2, space="PSUM") as opp,
            tc.tile_pool(name="fpp", bufs=2, space="PSUM") as fpp,
        ):
            # ---- weights, resident for the whole kernel ----
            wq_t = wp.tile([128, KT_Q, INNER], bf, tag="wq")
            wo_t = wp.tile([128, KT_Q, DQ], bf, tag="wo")
            wk_t = wp.tile([128, KT_C, INNER], bf, tag="wk")
            wv_t = wp.tile([128, KT_C, INNER], bf, tag="wv")
            bo_t = wp.tile([128, DQ // 128], f32, tag="bo")
            nc.sync.dma_start(wq_t[:], wq.rearrange("(k p) n -> p k n", p=128))
            nc.sync.dma_start(wo_t[:], wo.rearrange("(k p) n -> p k n", p=128))
            nc.sync.dma_start(wk_t[:], wk.rearrange("(k p) n -> p k n", p=128))
            nc.sync.dma_start(wv_t[:], wv.rearrange("(k p) n -> p k n", p=128))
            nc.sync.dma_start(bo_t[:], bo2[:, :])

            for b in range(BLOCAL):
                # ---- K/V projections for this batch ----
                ctx_t = kvp.tile([128, KT_C, NC], bf, tag="ctx")
                nc.sync.dma_start(
                    ctx_t[:], ctxT[b].rearrange("(k p) n -> p k n", p=128)
                )
                kT_t = kvp.tile([128, KT_Q, NC], bf, tag="kT")
                for c in range(KT_Q):
                    kps = qpp.tile([128, NC], f32, tag="qp")
                    for j in range(KT_C):
                        nc.tensor.matmul(
                            kps[:],
                            wk_t[:, j, c * 128:(c + 1) * 128],
                            ctx_t[:, j, :],
                            start=(j == 0),
                            stop=(j == KT_C - 1),
                        )
                    nc.any.tensor_copy(kT_t[:, c, :], kps[:])
                vaug = kvp.tile([NC, HEADS * VAUG], bf, tag="vaug")
                for n in range(2):
                    vps = spp.tile([NC, 512], f32, tag="sp")
                    for j in range(KT_C):
                        nc.tensor.matmul(
                            vps[:],
                            ctx_t[:, j, :],
                            wv_t[:, j, n * 512:(n + 1) * 512],
                            start=(j == 0),
                            stop=(j == KT_C - 1),
                        )
                    for hh in range(8):
                        h = n * 8 + hh
                        nc.any.tensor_copy(
                            vaug[:, h * VAUG:h * VAUG + DH],
                            vps[:, hh * DH:(hh + 1) * DH],
                        )
                for h in range(HEADS):
                    nc.vector.memset(vaug[:, h * VAUG + DH:(h + 1) * VAUG], 1.0)

                for blk in range(NBLK):
                    q0 = blk * BLK
                    # ---- load x^T block (one DMA) ----
                    xt = xtp.tile([128, KT_Q, BLK], bf, tag="xt")
                    nc.sync.dma_start(
                        xt[:],
                        xT[b].rearrange("(k p) n -> p k n", p=128)[:, :, q0:q0 + BLK],
                    )
                    # ---- Q projection: q^T block, bf16 ----
                    qt = []
                    for c in range(KT_Q):
                        qps = qpp.tile([128, BLK], f32, tag="qp")
                        for k in range(KT_Q):
                            nc.tensor.matmul(
                                qps[:],
                                wq_t[:, k, c * 128:(c + 1) * 128],
                                xt[:, k, :],
                                start=(k == 0),
                                stop=(k == KT_Q - 1),
                            )
                        q = qtp.tile([128, BLK], bf, tag="qt")
                        nc.any.tensor_copy(q[:], qps[:])
                        qt.append(q)

                    # ---- attention (all 16 heads) ----
                    zb = zbp.tile([HEADS, BLK], f32, tag="zb")
                    otz = []
                    for h in range(HEADS):
                        c, half = divmod(h, 2)
                        kh = kT_t[:, c, :][half * 64:(half + 1) * 64, :]
                        qh = qt[c][half * 64:(half + 1) * 64, :]
                        sps = spp.tile([NC, BLK], f32, tag="sp")
                        nc.tensor.matmul(sps[:], kh, qh, start=True, stop=True)
                        pt = ptp.tile([NC, BLK], bf, tag="pt")
                        nc.scalar.activation(pt[:], sps[:], AF.Exp)
                        ops = opp.tile([VAUG, BLK], f32, tag="op")
                        nc.tensor.matmul(
                            ops[:],
                            vaug[:, h * VAUG:(h + 1) * VAUG],
                            pt[:],
                            start=True,
                            stop=True,
                        )
                        oz = otzp.tile([VAUG, BLK], f32, tag="otz")
                        nc.any.tensor_copy(oz[:], ops[:])
                        nc.sync.dma_start(zb[h:h + 1, :], oz[DH:DH + 1, :])
                        otz.append(oz)

                    # ---- softmax denominators: invZ = exp(-ln Z) ----
                    lnz = zbp.tile([HEADS, BLK], f32, tag="lnz")
                    nc.scalar.activation(lnz[:], zb[:], AF.Ln)
                    izb = zbp.tile([HEADS, BLK], f32, tag="izb")
                    nc.scalar.activation(izb[:], lnz[:], AF.Exp, scale=-1.0)

                    # ---- normalize into assembled out^T (bf16) ----
                    ot = [
                        otp.tile([128, BLK], bf, tag="ot", name=f"ot{c}")
                        for c in range(KT_Q)
                    ]
                    for h in range(HEADS):
                        c, half = divmod(h, 2)
                        izr = izrp.tile([DH, BLK], f32, tag="izr")
                        nc.sync.dma_start(
                            izr[:],
                            izb[h:h + 1, :].unsqueeze(1).broadcast_to([1, DH, BLK]),
                        )
                        nc.gpsimd.tensor_mul(
                            ot[c][half * 64:(half + 1) * 64, :],
                            otz[h][0:DH, :],
                            izr[:],
                        )

                    # ---- output projection: y^T block ----
                    fo = fop.tile([128, KT_Q, BLK], f32, tag="fo")
                    for c2 in range(KT_Q):
                        fps = fpp.tile([128, BLK], f32, tag="fp")
                        for k in range(KT_Q):
                            nc.tensor.matmul(
                                fps[:],
                                wo_t[:, k, c2 * 128:(c2 + 1) * 128],
                                ot[k][:],
                                start=(k == 0),
                                stop=(k == KT_Q - 1),
                            )
                        nc.vector.tensor_scalar_add(
                            fo[:, c2, :], fps[:], bo_t[:, c2:c2 + 1]
                        )
                    nc.sync.dma_start(
                        outT[b].rearrange("(c p) n -> p c n", p=128)[:, :, q0:q0 + BLK],
                        fo[:],
                    )

    nc.compile()
    return nc


def _get_nc():
    if "nc" not in _cache:
        _cache["nc"] = _build()
    return _cache["nc"]


def kernel(x, context, Wq, Wk, Wv, Wo, bo):
    import ml_dtypes
    from concourse.bass_utils import run_bass_kernel_spmd

    bf16 = ml_dtypes.bfloat16
    x = np.asarray(x, dtype=np.float32)
    context = np.asarray(context, dtype=np.float32)
    Wq = np.asarray(Wq, dtype=np.float32)
    Wk = np.asarray(Wk, dtype=np.float32)
    Wv = np.asarray(Wv, dtype=np.float32)
    Wo = np.asarray(Wo, dtype=np.float32)
    bo = np.asarray(bo, dtype=np.float32)

    wq_b = np.ascontiguousarray(Wq * SCALE).astype(bf16)   # fold 1/sqrt(dh)
    wk_b = np.ascontiguousarray(Wk).astype(bf16)
    wv_b = np.ascontiguousarray(Wv).astype(bf16)
    wo_b = np.ascontiguousarray(Wo).astype(bf16)
    bo2 = np.ascontiguousarray(bo.reshape(DQ // 128, 128).T).astype(np.float32)

    in_maps = []
    for i in range(NCORES):
        sl = slice(i * BLOCAL, (i + 1) * BLOCAL)
        in_maps.append({
            "xT": np.ascontiguousarray(x[sl].transpose(0, 2, 1)).astype(bf16),
            "ctxT": np.ascontiguousarray(context[sl].transpose(0, 2, 1)).astype(bf16),
            "wq": wq_b,
            "wk": wk_b,
            "wv": wv_b,
            "wo": wo_b,
            "bo2": bo2,
        })

    nc = _get_nc()
    res = run_bass_kernel_spmd(
        nc, in_maps, core_ids=list(range(NCORES)),
        trace=bool(os.environ.get("KRN_TRACE")),
    )
    kernel._last_results = res

    out = np.empty((B, NQ, DQ), np.float32)
    for i in range(NCORES):
        out[i * BLOCAL:(i + 1) * BLOCAL] = res.results[i]["outT"].transpose(0, 2, 1)
    return out
